# revision 1
# baseline (speedup 1.0000x reference)
"""Trainium2 Bass kernel for the blob-layer problem.

Computes out[b, c] = sum_hw x[b, hw] * curves[hw, c] / (H*W) where
curves[hw, c] = clip(factor_c * exp(-((xs-px_c)^2 + (ys-py_c)^2)/s2_c) * w_c).

Strategy (8 NeuronCores, SPMD):
- Shard the pixel (contraction) dim into 8 y-bands of 28 rows; each core
  computes a partial (B, C) output which the host sums.
- Per core, prune curve columns whose blob cannot reach its band
  (|py - band| > sqrt(T*s2)); contributions beyond that are < e^-T ~ 1e-11.
- grid is a rank-4 bilinear form:
    -grid = 2*px*xs + 2*py*ys - (px^2+py^2) - (xs^2+ys^2)
  so a K=4 fp32 matmul produces G = -grid for a 128-pixel tile against all
  kept columns. A DVE multiply by a replicated 1/s2 row gives M = -grid/s2
  (exact fp32; reduced-precision matmul is NOT usable here because 1/s2
  amplifies absolute error up to 1000x). ScalarE Exp produces e in bf16,
  and bf16 matmuls accumulate out[b, c] += x[hw, b] * e[hw, c] in PSUM.
- The clip never binds when max|factor*w| <= CAP (e <= 1), which holds for
  these inputs, so factor*w/npix is folded into a final per-column scale.
"""
import os
import sys

sys.path.insert(0, "/opt/trn_rl_repo")

import ml_dtypes
import numpy as np

import concourse.bass as bass
import concourse.bacc as bacc
import concourse.tile as tile
from concourse import mybir
from concourse.bass_utils import run_bass_kernel_spmd

H, W, B, C = 224, 224, 256, 1024
NDEV = 8
ROWS = H // NDEV          # 28 rows per band
HWD = ROWS * W            # 6272 pixels per band
NT = HWD // 128           # 49 pixel tiles per band
EPS = 0.001
CAP = 2000.0
NPIX = float(H * W)
T_PRUNE = 25.0            # exp(-25) ~ 1.4e-11: dropped-column contribution bound

last_results = None       # BassKernelResults of the most recent run (for profiling)


def _build_program(nc_cols, reps=1, hw_loop=False, skip_g=False, skip_main=False,
                   skip_act=False, skew=2):
    """Emit the SPMD Bass program for NC kept/padded columns per core.

    Sync-wait discipline: a fused fp32 LDWEIGHTS+MATMUL accepts only ONE
    semaphore wait, so every fp32 matmul may depend on at most one new tick.
    The A matrix is pre-scaled by 1/s2 so the K=4 fp32 matmul yields
    M = -grid/s2 directly in PSUM and ScalarE's Exp reads PSUM — no DVE
    stage. The G matmul's only dep is then a single PE sem value (PSUM slot
    release merged with the weight-register WAR); its ACT release is already
    observed via the preceding main matmul's e-wait. bf16 main matmuls get a
    split LDWEIGHTS, so their x-DMA wait and e-ACT wait land on separate
    instructions. Bm/Am share one DMA so the first G matmul sees one queue
    sem.
    """
    NC = nc_cols
    nc = bacc.Bacc()
    f32 = mybir.dt.float32
    f16 = mybir.dt.float16
    bf16 = mybir.dt.bfloat16

    d_xT = nc.declare_dram_parameter("xT", [NT, 128, B], bf16, isOutput=False)
    d_Wp = nc.declare_dram_parameter("Wp", [12, HWD], f16, isOutput=False)
    d_Mv = nc.declare_dram_parameter("Mv", [12, NC], f16, isOutput=False)
    d_Fw = nc.declare_dram_parameter("Fw", [128, NC], f32, isOutput=False)
    d_out = nc.declare_dram_parameter("out", [2, 128, NC], f32, isOutput=True)

    c_chunks = [(c0, min(512, NC - c0)) for c0 in range(0, NC, 512)]

    with tile.TileContext(nc) as tc:
        with (
            tc.tile_pool(name="const", bufs=1) as cpool,
            tc.tile_pool(name="ep", bufs=4) as ep,
            tc.tile_pool(name="op", bufs=1) as op,
            tc.tile_pool(name="psG", bufs=4, space="PSUM") as psG,
            tc.tile_pool(name="psO", bufs=1, space="PSUM") as psO,
        ):
            Wp = cpool.tile([12, HWD], f16, tag="Wp")
            Mv = cpool.tile([12, NC], f16, tag="Mv")
            Fw = cpool.tile([128, NC], f32, tag="Fw")
            nc.gpsimd.dma_start(Wp[:], d_Wp[:])
            nc.gpsimd.dma_start(Mv[:], d_Mv[:])
            nc.gpsimd.dma_start(Fw[:], d_Fw[:])

            # whole x band stays SBUF-resident (25KB/partition): a few large
            # DMAs write disjoint ranges of one tile, so no slot-recycle or
            # queue-ring waits exist and each main LDWEIGHTS waits on at most
            # one DMA queue sem.
            xfull = cpool.tile([128, NT * B], bf16, tag="xfull")
            grp = (NT + 6) // 7
            for t0 in range(0, NT, grp):
                t1 = min(t0 + grp, NT)
                nc.sync.dma_start(
                    xfull[:, t0 * B : t1 * B].rearrange(
                        "p (t b) -> p t b", t=t1 - t0
                    ),
                    d_xT[t0:t1].rearrange("t p b -> p t b"),
                )

            Op0 = psO.tile([128, NC], f32, tag="op0")
            Op1 = psO.tile([128, NC], f32, tag="op1")

            def emit_main(j, e):
                first, last = j == 0, j == NT - 1
                for bb, Opx in ((0, Op0), (1, Op1)):
                    for c0, cw in c_chunks:
                        nc.tensor.matmul(
                            Opx[:, c0 : c0 + cw],
                            xfull[:, j * B + bb * 128 : j * B + (bb + 1) * 128],
                            e[:, c0 : c0 + cw],
                            start=first,
                            stop=last,
                            skip_group_check=True,
                        )

            def one_pass():
                pending = []
                for t in range(NT):
                    Gp = psG.tile([128, NC], f32, tag="Gp")
                    if not skip_g:
                        # M = -grid/s2 via 3-term fp16 hi/lo split (hi*hi +
                        # hi*lo + lo*hi ~ 2^-22 precision; fp16 runs at full
                        # PE rate while fp32 measured ~8x slower; subnormals
                        # honored — probed). The three K=4 terms are STACKED
                        # along the contraction dim (K=12, rows [Bh;Bh;Bl] x
                        # [Ah;Al;Ah]) so one matmul computes the whole sum at
                        # the cost of a K=4 one.
                        for c0, cw in c_chunks:
                            nc.tensor.matmul(
                                Gp[:, c0 : c0 + cw],
                                Wp[:, t * 128 : (t + 1) * 128],
                                Mv[:, c0 : c0 + cw],
                                start=True,
                                stop=True,
                                skip_group_check=True,
                            )
                    e = ep.tile([128, NC], bf16, tag="e")
                    if skip_act:
                        if t < skew + 1:
                            nc.scalar.activation(
                                e[:], Gp[:], mybir.ActivationFunctionType.Exp
                            )
                    else:
                        nc.scalar.activation(
                            e[:], Gp[:], mybir.ActivationFunctionType.Exp
                        )

                    pending.append((t, e))
                    if len(pending) > skew and not skip_main:
                        emit_main(*pending.pop(0))
                if not skip_main:
                    while pending:
                        emit_main(*pending.pop(0))

            # reps>1 repeats the identical computation (timing harness only;
            # each rep's start=True resets the accumulators, so the final
            # output is unchanged).
            if hw_loop and reps > 1:
                with tc.For_i(0, reps, 1):
                    one_pass()
            else:
                for _ in range(reps):
                    one_pass()

            out_sb = op.tile([128, 2 * NC], f32, tag="out")
            nc.vector.tensor_mul(out_sb[:, 0:NC], Op0[:], Fw[:])
            nc.vector.tensor_mul(out_sb[:, NC : 2 * NC], Op1[:], Fw[:])
            nc.sync.dma_start(d_out[0], out_sb[:, 0:NC])
            nc.sync.dma_start(d_out[1], out_sb[:, NC : 2 * NC])

    nc.compile()
    _dedup_ldweights(nc)
    return nc


def _dedup_ldweights(nc):
    """Drop LDWEIGHTS that reload the exact weights already resident in the
    PE array (same AP, no intervening load, no sems). ~107ns each on HW; the
    cost model prices them at 0 so Tile never minimizes them."""
    for f in nc.m.functions:
        for blk in f.blocks:
            keep = []
            prev_ap = None
            for inst in blk.instructions:
                tn = type(inst).__name__
                if tn == "InstLdweights":
                    si = inst.sync_info
                    w = (si.on_wait if si else []) or []
                    u = (si.on_update if si else []) or []
                    ap = repr(inst.ins[0])
                    if ap == prev_ap and not w and not u:
                        continue
                    prev_ap = ap
                keep.append(inst)
            if len(keep) != len(blk.instructions):
                del blk.instructions[:]
                blk.instructions.extend(keep)


def _prepare(x, positions, sigmas, curve_weights, xs, ys):
    x = np.asarray(x, dtype=np.float32)
    px = np.asarray(positions, dtype=np.float64)[0, 0, :, 1]
    py = np.asarray(positions, dtype=np.float64)[0, 0, :, 0]
    sg = np.asarray(sigmas, dtype=np.float64)[0, 0]
    w = np.asarray(curve_weights, dtype=np.float64)[0, 0]
    xs = np.asarray(xs, dtype=np.float64)
    ys = np.asarray(ys, dtype=np.float64)

    s2 = 2.0 * sg * sg + EPS
    factor = 1.0 / (2.0 * np.pi * sg * sg + EPS)
    fw = factor * w
    # clip(curves) is identity when max|factor*w| <= CAP since exp(...) <= 1
    assert np.abs(fw).max() <= CAP, "clip binds; folded-scale scheme invalid"

    # Per band keep columns whose blob reaches it (margin^2/s2 <= T), capped
    # at 512 (one PSUM bank) by dropping the weakest-coupled columns; the
    # implied threshold of dropped columns stays >= ~10 (contribution < 1e-4
    # relative).
    keep_idx = []
    for d in range(NDEV):
        h0 = d * ROWS
        y0, y1 = ys[h0, 0], ys[h0 + ROWS - 1, 0]
        margin = np.maximum(np.maximum(y0 - py, py - y1), 0.0)
        score = margin * margin / s2
        idx = np.where(score <= T_PRUNE)[0]
        if len(idx) > 512:
            idx = idx[np.argsort(score[idx], kind="stable")[:512]]
            idx.sort()
        keep_idx.append(idx)
    NC = max(128, -(-max(len(i) for i in keep_idx) // 128) * 128)
    assert NC <= 512

    in_maps = []
    for d in range(NDEV):
        h0 = d * ROWS
        rows = slice(h0, h0 + ROWS)
        xs_b = xs[rows].ravel()
        ys_b = ys[rows].ravel()
        Bm = np.stack(
            [xs_b, ys_b, np.ones(HWD), xs_b * xs_b + ys_b * ys_b]
        ).astype(np.float32)

        idx = keep_idx[d]
        nk = len(idx)
        # A columns pre-scaled by 1/s2 so the matmul yields M = -grid/s2
        Am = np.zeros((4, NC), np.float32)
        Am[0, :nk] = 2.0 * px[idx] / s2[idx]
        Am[1, :nk] = 2.0 * py[idx] / s2[idx]
        Am[2, :nk] = -(px[idx] ** 2 + py[idx] ** 2) / s2[idx]
        Am[3, :nk] = -1.0 / s2[idx]
        Am[3, nk:] = -1.0
        Bm64 = Bm.astype(np.float64)
        Am64 = Am.astype(np.float64)
        Bh = Bm64.astype(np.float16)
        Bl = (Bm64 - Bh.astype(np.float64)).astype(np.float16)
        Ah = Am64.astype(np.float16)
        Al = (Am64 - Ah.astype(np.float64)).astype(np.float16)
        # K=12 stacked hi/lo split: [Bh;Bh;Bl]^T @ [Ah;Al;Ah]
        Wp = np.concatenate([Bh, Bh, Bl], axis=0)
        Mv = np.concatenate([Ah, Al, Ah], axis=0)
        F = np.zeros(NC, np.float64)
        F[:nk] = fw[idx] / NPIX

        xT = np.ascontiguousarray(
            x[:, rows, :].reshape(B, HWD).T
        ).reshape(NT, 128, B).astype(ml_dtypes.bfloat16)

        in_maps.append(
            {
                "xT": xT,
                "Wp": Wp,
                "Mv": Mv,
                "Fw": np.ascontiguousarray(
                    np.broadcast_to(F.astype(np.float32), (128, NC))
                ),
            }
        )
    return NC, in_maps, keep_idx


def _gather(results, keep_idx, NC):
    out = np.zeros((B, C), np.float32)
    for d in range(NDEV):
        nk = len(keep_idx[d])
        dev = np.asarray(results[d]["out"], np.float32).reshape(B, NC)
        out[:, keep_idx[d]] += dev[:, :nk]
    return out


def kernel(x, positions, sigmas, curve_weights, xs, ys):
    global last_results
    NC, in_maps, keep_idx = _prepare(x, positions, sigmas, curve_weights, xs, ys)
    nc = _build_program(NC)
    trace = bool(os.environ.get("BLOB_TRACE"))
    last_results = run_bass_kernel_spmd(
        nc, in_maps, list(range(NDEV)), trace=trace
    )
    return _gather(last_results.results, keep_idx, NC)



# revision 2
# speedup vs baseline: 2.1308x; 2.1308x over previous
"""Trainium2 Bass kernel for the blob-layer problem.

Computes out[b, c] = sum_hw x[b, hw] * curves[hw, c] / (H*W) where
curves[hw, c] = clip(factor_c * exp(-((xs-px_c)^2 + (ys-py_c)^2)/s2_c) * w_c).

Strategy (8 NeuronCores, SPMD):
- The Gaussian is SEPARABLE: exp(-((x-px)^2+(y-py)^2)/s2) =
  Ex[w,c] * Ey[h,c]. Host computes the tiny 1-D tables (O((H+W)*C)
  exps); the device builds each 128-pixel tile's exp field with ONE
  DVE multiply of partition-replicated table tiles — no G matmul, no
  ACT Exp. The tensor engine does ONLY the main contraction.
- 2D core grid (4 y-bands x 2 x-halves), block = 56x112 pixels per
  core, tiled 7x7 as (8 rows x 16 cols) = 128-pixel tiles. 2D pruning
  keeps columns whose blob reaches the block (corner distance^2/s2 <=
  T=9), capped at NC=320 by dropping weakest-coupled columns
  (measured pruning-only rel err 1.7e-3 vs the 2e-2 gate).
- Per tile t=(i,j): e = EyR_i * ExR_j (DVE, fp16), then 2 fp16
  matmuls accumulate out[b, c] += x[p, b] * e[p, c] into PSUM (one
  per 128-batch chunk). fp16 throughout beats the old bf16 on
  precision (2^-11 vs 2^-8) at the same PE rate.
- Keeping the PE continuously busy matters beyond occupancy: TRN2
  ramps the PE clock 0.65 -> 1.2 -> 2.4 GHz and only reaches max
  after ~3us of uninterrupted execution.
- factor*w/npix and the column gather/unpad are applied on the host
  during the final (B, C) assembly; the clip never binds when
  max|factor*w| <= CAP (exp <= 1), which is asserted.
"""
import os
import sys

sys.path.insert(0, "/opt/trn_rl_repo")

import numpy as np

import concourse.bass as bass
import concourse.bacc as bacc
import concourse.tile as tile
from concourse import mybir
from concourse.bass_utils import run_bass_kernel_spmd

H, W, B, C = 224, 224, 256, 1024
NDEV = 8
GY, GX = 4, 2             # core grid: 4 y-bands x 2 x-halves
BY, BX = H // GY, W // GX  # 56 x 112 block per core
TY, TX = 8, 16            # tile = 8 rows x 16 cols = 128 pixels
NI, NJ = BY // TY, BX // TX  # 7 x 7 tiles
NT = NI * NJ              # 49 tiles
NC = 320                  # kept/padded columns per core
EPS = 0.001
CAP = 2000.0
NPIX = float(H * W)
T_PRUNE = 9.0

last_results = None       # BassKernelResults of the most recent run (for profiling)


def _build_program(reps=1, skew=2):
    """Emit the SPMD Bass program: per tile one DVE multiply builds the
    exp field from replicated 1-D tables, then two fp16 matmuls
    accumulate the 256-batch contraction into two PSUM banks."""
    nc = bacc.Bacc()
    f32 = mybir.dt.float32
    f16 = mybir.dt.float16

    d_xT = nc.declare_dram_parameter("xT", [NT, 128, B], f16, isOutput=False)
    d_EyR = nc.declare_dram_parameter("EyR", [128, NI, NC], f16, isOutput=False)
    d_ExR = nc.declare_dram_parameter("ExR", [128, NJ, NC], f16, isOutput=False)
    d_out = nc.declare_dram_parameter("out", [2, 128, NC], f16, isOutput=True)

    with tile.TileContext(nc) as tc:
        with (
            tc.tile_pool(name="const", bufs=1) as cpool,
            tc.tile_pool(name="ep", bufs=4) as ep,
            tc.tile_pool(name="op", bufs=1) as op,
            tc.tile_pool(name="psO", bufs=1, space="PSUM") as psO,
        ):
            EyR = cpool.tile([128, NI * NC], f16, tag="EyR")
            ExR = cpool.tile([128, NJ * NC], f16, tag="ExR")
            # interleave so the earliest-needed chunks land first
            for k in range(NI):
                nc.gpsimd.dma_start(EyR[:, k * NC : (k + 1) * NC], d_EyR[:, k, :])
                nc.gpsimd.dma_start(ExR[:, k * NC : (k + 1) * NC], d_ExR[:, k, :])

            # whole x block stays SBUF-resident (24.5KB/partition); a few
            # large DMAs write disjoint ranges of one tile so each matmul
            # LDWEIGHTS waits on at most one DMA queue sem.
            xfull = cpool.tile([128, NT * B], f16, tag="xfull")
            grp = (NT + 6) // 7
            for t0 in range(0, NT, grp):
                t1 = min(t0 + grp, NT)
                nc.sync.dma_start(
                    xfull[:, t0 * B : t1 * B].rearrange(
                        "p (t b) -> p t b", t=t1 - t0
                    ),
                    d_xT[t0:t1].rearrange("t p b -> p t b"),
                )

            # PSUM accumulators: full-bank tiles so each is bank-aligned
            Op0 = psO.tile([128, 512], f32, tag="op0")
            Op1 = psO.tile([128, 512], f32, tag="op1")

            def emit_main(t, e):
                first, last = t == 0, t == NT - 1
                for bb, Opx in ((0, Op0), (1, Op1)):
                    nc.tensor.matmul(
                        Opx[:, 0:NC],
                        xfull[:, t * B + bb * 128 : t * B + (bb + 1) * 128],
                        e[:],
                        start=first,
                        stop=last,
                        skip_group_check=True,
                    )

            def one_pass():
                pending = []
                for t in range(NT):
                    i, j = t // NJ, t % NJ
                    e = ep.tile([128, NC], f16, tag="e")
                    nc.vector.tensor_mul(
                        e[:],
                        EyR[:, i * NC : (i + 1) * NC],
                        ExR[:, j * NC : (j + 1) * NC],
                    )
                    pending.append((t, e))
                    if len(pending) > skew:
                        emit_main(*pending.pop(0))
                while pending:
                    emit_main(*pending.pop(0))

            # reps>1 repeats the identical computation (timing harness only;
            # each rep's start=True resets the accumulators).
            for _ in range(reps):
                one_pass()

            out_sb = op.tile([128, 2 * NC], f16, tag="out")
            nc.scalar.activation(
                out_sb[:, 0:NC], Op0[:, 0:NC], mybir.ActivationFunctionType.Copy
            )
            nc.scalar.activation(
                out_sb[:, NC : 2 * NC], Op1[:, 0:NC],
                mybir.ActivationFunctionType.Copy,
            )
            nc.sync.dma_start(d_out[0], out_sb[:, 0:NC])
            nc.sync.dma_start(d_out[1], out_sb[:, NC : 2 * NC])

    nc.compile()
    return nc


def _prepare(x, positions, sigmas, curve_weights, xs, ys):
    x = np.asarray(x, dtype=np.float32)
    px = np.asarray(positions, dtype=np.float64)[0, 0, :, 1]
    py = np.asarray(positions, dtype=np.float64)[0, 0, :, 0]
    sg = np.asarray(sigmas, dtype=np.float64)[0, 0]
    w = np.asarray(curve_weights, dtype=np.float64)[0, 0]
    xs = np.asarray(xs, dtype=np.float64)
    ys = np.asarray(ys, dtype=np.float64)

    # separability requires xs constant along rows, ys along cols
    assert np.allclose(xs, xs[0:1, :]) and np.allclose(ys, ys[:, 0:1])
    xs_ax = xs[0, :]
    ys_ax = ys[:, 0]

    s2 = 2.0 * sg * sg + EPS
    factor = 1.0 / (2.0 * np.pi * sg * sg + EPS)
    fw = factor * w
    # clip(curves) is identity when max|factor*w| <= CAP since exp(...) <= 1
    assert np.abs(fw).max() <= CAP, "clip binds; folded-scale scheme invalid"

    in_maps = []
    keep_idx = []
    for d in range(NDEV):
        iy, ix = d // GX, d % GX
        y0, x0 = iy * BY, ix * BX
        rows = ys_ax[y0 : y0 + BY]
        cols = xs_ax[x0 : x0 + BX]

        # 2D prune: closest-corner distance^2 / s2, cap at NC
        my = np.maximum(np.maximum(rows[0] - py, py - rows[-1]), 0.0)
        mx = np.maximum(np.maximum(cols[0] - px, px - cols[-1]), 0.0)
        score = (my * my + mx * mx) / s2
        idx = np.where(score <= T_PRUNE)[0]
        if len(idx) > NC:
            idx = idx[np.argsort(score[idx], kind="stable")[:NC]]
            idx.sort()
        nk = len(idx)
        keep_idx.append(idx)

        # 1-D exp tables over the block's rows/cols, kept columns only
        Ey = np.zeros((BY, NC), np.float64)
        Ex = np.zeros((BX, NC), np.float64)
        Ey[:, :nk] = np.exp(-((rows[:, None] - py[idx]) ** 2) / s2[idx])
        Ex[:, :nk] = np.exp(-((cols[:, None] - px[idx]) ** 2) / s2[idx])

        # partition-replicated tile factors: partition l = r*TX + wi
        # EyR[l, i, c] = Ey[TY*i + l//TX, c]; ExR[l, j, c] = Ex[TX*j + l%TX, c]
        EyR = np.ascontiguousarray(
            np.broadcast_to(
                Ey.reshape(NI, TY, 1, 1, NC), (NI, TY, TX, 1, NC)
            ).transpose(1, 2, 0, 3, 4).reshape(128, NI, NC)
        ).astype(np.float16)
        ExR = np.ascontiguousarray(
            np.broadcast_to(
                Ex.reshape(1, NJ, 1, TX, NC), (TY, NJ, 1, TX, NC)
            ).transpose(0, 3, 1, 2, 4).reshape(128, NJ, NC)
        ).astype(np.float16)

        # x tile layout: xT[t=(i*NJ+j), l=(r*TX+wi), b] = x[b, y0+TY*i+r, x0+TX*j+wi]
        xb = x[:, y0 : y0 + BY, x0 : x0 + BX]
        xT = np.ascontiguousarray(
            xb.reshape(B, NI, TY, NJ, TX).transpose(1, 3, 2, 4, 0).reshape(NT, 128, B)
        ).astype(np.float16)

        in_maps.append({"xT": xT, "EyR": EyR, "ExR": ExR})
    return in_maps, keep_idx, fw


def _gather(results, keep_idx, fw):
    out = np.zeros((B, C), np.float32)
    for d in range(NDEV):
        idx = keep_idx[d]
        nk = len(idx)
        dev = np.asarray(results[d]["out"], np.float32).reshape(B, NC)
        out[:, idx] += dev[:, :nk] * (fw[idx] / NPIX).astype(np.float32)
    return out


def kernel(x, positions, sigmas, curve_weights, xs, ys):
    global last_results
    in_maps, keep_idx, fw = _prepare(x, positions, sigmas, curve_weights, xs, ys)
    nc = _build_program()
    trace = bool(os.environ.get("BLOB_TRACE"))
    last_results = run_bass_kernel_spmd(
        nc, in_maps, list(range(NDEV)), trace=trace
    )
    return _gather(last_results.results, keep_idx, fw)
